# revision 68
# baseline (speedup 1.0000x reference)
"""Causal self-attention on 8 TRN2 NeuronCores, batch-data-parallel (one batch
element per core).

Layout strategy (per core, S=1024, D=1024, H=16, hd=64):
  - Host pre-transposes x -> xT [D,S] and all weights -> [in_dim, out_dim],
    all converted to bf16 (same 1 col/cycle PE rate as fp32r but much lower
    PE power draw -- sustained fp32r trips the ~200us thermal firmware loop
    which gates the PE clock to 4/8 for the rest of the run).
  - qk projection produces q,k transposed ([e,s]) per head-pair: lhsT = wqkT
    tiles, rhs = xT.  Head h lives at partitions 64*(h%2)..+64.
  - v natural [s,e]: lhsT = xT tiles, rhs = resident wv half tiles; stored
    interleaved with a ones column per head (65 cols/head) so the AV matmul's
    PSUM row 64 is the softmax denominator (rowsum of unnormalized attn).
  - scoresT [sk,sq] per head-pair via K=64 matmuls; exp on ACT (scale=1/8
    folded in); causal diag masked by multiplicative upper-tri mask (DVE);
    fully-masked tiles never computed; AV rhs ranges narrowed to the
    causally-written columns (no zero-fills needed).
  - AV: outT'[hd+1, sq] accumulated m-major; normalization via
    approx-reciprocal of the den row (fp32) + PE rank-1 broadcast + DVE
    multiply (writes bf16 outT).
  - proj: y[s,e] with lhsT = outT tiles, rhs = wpT (DMA'd over the dead xT
    SBUF space during pairs 6-7) + rank-1 bias term (beff = b_proj +
    W_proj @ b_v; b_v folds exactly through softmax rowsum).  ko 0..2 are
    computed as partial sums interleaved into pairs 3-7 (tail PE filler),
    ko 3..7 finish after the loop with a DVE add of the DRAM partial.
  - QKV matmul quanta are interleaved into the attention pair loop; weight
    DMAs are hoisted 1+ pairs ahead of their consuming matmuls.
"""

import numpy as np

B, S, D, H = 8, 1024, 1024, 16
HD = D // H          # 64
P = 128
NCORES = 8
KO = D // P          # 8 contraction tiles over d
MT = (2 * D) // P    # 16 m-tiles for q,k
ST = S // P          # 8 s-tiles
NPAIRS = H // 2      # 8 head pairs

_CACHE = {}
TRACE = False        # set by test harness to collect an NTFF profile


def _score_chunks(w):
    # split w into <=512-wide pieces (PSUM bank limit); bf16 runs at full
    # rate at any width
    table = {1024: [512, 512], 896: [512, 384], 768: [512, 256],
             640: [384, 256], 512: [512], 384: [384], 256: [256], 128: [128]}
    return table[w]


def _build():
    import concourse.tile as tile
    from concourse import bacc, mybir

    F32R = mybir.dt.float32r
    F32 = mybir.dt.float32
    BF16 = mybir.dt.bfloat16
    AF = mybir.ActivationFunctionType

    nc = bacc.Bacc("TRN2", target_bir_lowering=False, debug=False,
                   num_devices=NCORES)
    xT_d = nc.dram_tensor("xT", [D, S], BF16, kind="ExternalInput").ap()
    # wqkT host-laid-out as [p, m, ko, e] so each m-tile's DMA reads
    # contiguous lines per partition
    wqkT_d = nc.dram_tensor("wqkT", [P, MT, KO, P], BF16,
                            kind="ExternalInput").ap()
    wvT_d = nc.dram_tensor("wvT", [D, D], BF16, kind="ExternalInput").ap()
    wpT_d = nc.dram_tensor("wpT", [D, D], BF16, kind="ExternalInput").ap()
    bqk_d = nc.dram_tensor("bqk", [2 * D], F32, kind="ExternalInput").ap()
    beff_d = nc.dram_tensor("beff", [D], F32R, kind="ExternalInput").ap()
    umask_d = nc.dram_tensor("umask", [P, P], F32, kind="ExternalInput").ap()
    y_d = nc.dram_tensor("y", [S, D], F32, kind="ExternalOutput").ap()

    wvT_v = wvT_d.rearrange("(ko p) e -> p ko e", p=P)
    wpT_v = wpT_d.rearrange("(ko p) e -> p ko e", p=P)
    xT_v = xT_d.rearrange("(ko p) s -> p ko s", p=P)

    with tile.TileContext(nc) as tc:
        with (
            tc.tile_pool(name="bigio", bufs=1) as bigio,
            tc.tile_pool(name="qkp", bufs=3) as qkp,
            tc.tile_pool(name="vp", bufs=1) as vpool,
            tc.tile_pool(name="wqk", bufs=4) as wqkp,
            tc.tile_pool(name="wvp", bufs=1) as wvp,
            tc.tile_pool(name="attn", bufs=6) as attnp,
            tc.tile_pool(name="rt", bufs=2) as rtp,
            tc.tile_pool(name="rb", bufs=1) as rbp,
            tc.tile_pool(name="todd", bufs=1) as toddp,
            tc.tile_pool(name="ypart", bufs=16) as ypartp,
            tc.tile_pool(name="avsb", bufs=2) as avsbp,
            tc.tile_pool(name="cst", bufs=1) as cst,
            tc.tile_pool(name="psS", bufs=4, space="PSUM") as psS,
            tc.tile_pool(name="psAV", bufs=4, space="PSUM") as psAV,
        ):
            # ---------- constants ----------
            umask = cst.tile([P, P], F32)
            bqk_sb = cst.tile([P, MT], F32)
            beff_sb = cst.tile([1, D], F32R)
            onecol = cst.tile([P, 1], F32)
            nc.vector.memset(onecol[:], 1.0)
            ones1x128 = cst.tile([1, P], F32R)
            nc.vector.tensor_copy(
                ones1x128[:], onecol[0:1, :].broadcast_to([1, P]))
            ones65r = cst.tile([65, 64], F32R)
            nc.vector.memset(ones65r[64:65, :].bitcast(F32), 1.0)

            # ---------- big SBUF residents ----------
            # xT is overwritten with wpT during pairs 6-7 (QKV matmuls are
            # done reading it by then); proj reads it as the wpT resident.
            xT = bigio.tile([P, KO, S], BF16, tag="xT")
            outT = bigio.tile([P, KO, S], BF16, tag="outT")
            v_sb = vpool.tile([P, ST, H * (HD + 1)], BF16)
            v_hview = v_sb[:].rearrange("p st (h c) -> p st h c", c=HD + 1)

            # ---------- input DMA schedule ----------
            wqk_tiles = {}

            def wqk_dma(j):
                for part in (0, 1):
                    m = j if part == 0 else NPAIRS + j
                    wt = wqkp.tile([P, KO, P], BF16, tag="wqk",
                                   name=f"wqk{m}")
                    nc.sync.dma_start(wt[:], wqkT_d[:, m, :, :])
                    wqk_tiles[m] = wt

            wv_res = {}

            def wv_dma(nE):
                wt = wvp.tile([P, KO, 512], BF16, tag="wv", name=f"wv{nE}")
                nc.sync.dma_start(wt[:],
                                  wvT_v[:, :, nE * 512:(nE + 1) * 512])
                wv_res[nE] = wt

            # critical-path DMAs first: the sync queue issues each dma_start
            # serially (~0.8us apiece), so the first quantum's inputs go
            # ahead of everything else
            # pair-0 q weights, then the xT wave; k weights (m=8) aren't
            # read until ~3.5us of q matmuls have run
            wt0 = wqkp.tile([P, KO, P], BF16, tag="wqk", name="wqk0")
            nc.sync.dma_start(wt0[:], wqkT_d[:, 0, :, :])
            wqk_tiles[0] = wt0
            nc.sync.dma_start(xT[:, 0, :], xT_v[:, 0, :])
            nc.sync.dma_start(xT[:, 1, :], xT_v[:, 1, :])
            wt8 = wqkp.tile([P, KO, P], BF16, tag="wqk", name="wqk8")
            nc.sync.dma_start(wt8[:], wqkT_d[:, NPAIRS, :, :])
            wqk_tiles[NPAIRS] = wt8
            nc.sync.dma_start(xT[:, 2, :], xT_v[:, 2, :])
            wqk_dma(1)
            for ko in range(3, KO):
                nc.sync.dma_start(xT[:, ko, :], xT_v[:, ko, :])
            wv_dma(0)
            nc.sync.dma_start(umask[:], umask_d)
            nc.sync.dma_start(bqk_sb[:], bqk_d.rearrange("(m p) -> p m", p=P))
            nc.sync.dma_start(beff_sb[:], beff_d[None, :])
            nc.vector.tensor_copy(
                v_hview[:, :, :, HD:HD + 1],
                onecol[:, None, None, :].broadcast_to([P, ST, H, 1]))

            qk_tiles = {}    # j -> [128, 2, S] tile (0=q, 1=k)

            # ---------- QKV work quanta (emitted interleaved) ----------
            def qk_quanta(j):
                # 4 closures; each computes one (part, nn) psum group.
                # Weights were DMA'd earlier via wqk_dma(j).
                t = qkp.tile([P, 2, S], BF16, tag="qkt", name=f"qk{j}")
                qk_tiles[j] = t

                def quantum(part, nn):    # part 0=q (m-tile j), 1=k (8+j)
                    def go():
                        m = j if part == 0 else NPAIRS + j
                        wt = wqk_tiles[m]
                        ps = psS.tile([P, 512], F32, tag="ps", name=f"qkps{m}")
                        for ko in range(KO):
                            nc.tensor.matmul(
                                ps[:], wt[:, ko, :],
                                xT[:, ko, nn * 512:(nn + 1) * 512],
                                start=(ko == 0), stop=(ko == KO - 1))
                        nc.vector.tensor_scalar_add(
                            t[:, part, nn * 512:(nn + 1) * 512], ps[:],
                            bqk_sb[:, m:m + 1])
                    return go
                return [quantum(0, 0), quantum(0, 1),
                        quantum(1, 0), quantum(1, 1)]

            def v_quanta(nE):
                # v half nE: e_v cols 512*nE.. (heads 8nE..8nE+7), 8 quanta
                # of 1 s-tile each, reading the resident wv half tile
                def quantum(st):
                    def go():
                        wt = wv_res[nE]
                        ps = psS.tile([P, 512], F32, tag="ps",
                                      name=f"vps{nE}_{st}")
                        for ko in range(KO):
                            nc.tensor.matmul(
                                ps[:], xT[:, ko, st * P:(st + 1) * P],
                                wt[:, ko, :], start=(ko == 0),
                                stop=(ko == KO - 1))
                        nc.vector.tensor_copy(
                            v_hview[:, st, 8 * nE:8 * (nE + 1), 0:HD],
                            ps[:].rearrange("p (h c) -> p h c", c=HD))
                    return go
                return [quantum(st) for st in range(ST)]

            # ---------- attention ----------
            pend = {}

            def scores_exp(j, m):
                qk_t = qk_tiles[j]
                w = S - m * P
                for hb, base in ((0, 0), (1, 64)):   # head 2j+hb
                    at = attnp.tile([P, S], BF16, tag="at",
                                    name=f"at{j}_{hb}_{m}")
                    pend[(j, hb, m)] = at
                    off = m * P
                    for cw in _score_chunks(w):
                        ps = psS.tile([P, 512], F32, tag="ps",
                                      name=f"sps{j}_{hb}_{m}")
                        nc.tensor.matmul(
                            ps[:, 0:cw],
                            qk_t[base:base + 64, 1, m * P:(m + 1) * P],
                            qk_t[base:base + 64, 0, off:off + cw],
                            start=True, stop=True)
                        nc.scalar.activation(
                            at[:, off:off + cw], ps[:, 0:cw], AF.Exp,
                            scale=0.125)
                        off += cw
                    nc.vector.tensor_mul(
                        at[:, m * P:(m + 1) * P], at[:, m * P:(m + 1) * P],
                        umask[:])

            def av_m(j, m):
                # rhs narrowed to the causally-written at columns, so the
                # at tiles need no zero-fill of the masked prefix.  The
                # accumulators are [65, 512] column halves: the n=0 half
                # closes at m=3 and is evicted mid-pair, so the next pair's
                # AV only ever waits on the n=1 eviction.
                st8 = pend[f"ps{j}"]
                for hb in (0, 1):
                    h = 2 * j + hb
                    at = pend[(j, hb, m)]
                    for n in range((0 if m <= 3 else 1), 2):
                        lo = max(n * 512, m * P)
                        nc.tensor.matmul(
                            st8[hb][n][:, lo - n * 512:512],
                            v_sb[:, m, h * (HD + 1):(h + 1) * (HD + 1)],
                            at[:, lo:(n + 1) * 512],
                            start=(m == 0), stop=(m == 4 * n + 3))

            def evict_n0(j):
                # n=0 halves are complete after av_m(j, 3); evicting them
                # early frees their PSUM banks for the next pair
                avcs = []
                for hb in (0, 1):
                    avc = avsbp.tile([65, S], F32R, tag="avc",
                                     name=f"avc{j}_{hb}")
                    eng = nc.scalar.copy if hb == 0 else nc.vector.tensor_copy
                    eng(avc[:, 0:512], pend[f"ps{j}"][hb][0][:])
                    avcs.append(avc)
                pend[f"avc{j}"] = avcs

            def evict_recip(j):
                # evict the n=1 halves, then take the reciprocal of the den
                # row (approx_fast: ~4e-6 rel; den >= exp(0) > 0)
                recs = []
                for hb in (0, 1):
                    avc = pend[f"avc{j}"][hb]
                    eng = nc.scalar.copy if hb == 1 else nc.vector.tensor_copy
                    eng(avc[:, 512:1024], pend[f"ps{j}"][hb][1][:])
                    rt = rtp.tile([65, S], F32R, tag="rt")
                    # custom-DVE op misbehaves on single-partition APs on HW:
                    # run it over all 65 rows (lanes are parallel) and consume
                    # only the den row (64); other lanes are never read
                    for c in range(4):
                        rt32 = rtp.tile([65, 256], F32, tag="rt32", bufs=1)
                        nc.vector.reciprocal_approx_fast(
                            rt32[:],
                            avc[:, c * 256:(c + 1) * 256].bitcast(F32))
                        nc.vector.tensor_copy(
                            rt[64:65, c * 256:(c + 1) * 256], rt32[64:65, :])
                    recs.append(rt)
                pend[f"rec{j}"] = recs
                del pend[f"ps{j}"]

            def rb_norm(j):
                for hb in (0, 1):
                    rt = pend[f"rec{j}"][hb]
                    rb_t = rbp.tile([64, S], F32R, tag="rb")
                    for c in range(2):
                        rps = psS.tile([P, 512], F32, tag="ps",
                                       name=f"rbps{j}_{hb}_{c}")
                        nc.tensor.matmul(
                            rps[0:64, :], ones65r[64:65, :],
                            rt[64:65, c * 512:(c + 1) * 512],
                            start=True, stop=True)
                        nc.scalar.copy(
                            rb_t[:, c * 512:(c + 1) * 512], rps[0:64, :])
                    avc = pend[f"avc{j}"][hb]
                    if hb == 0:
                        nc.vector.tensor_mul(
                            outT[0:64, j, :], avc[0:64, :], rb_t[:])
                    else:
                        # lanes cannot shift partitions: multiply to an
                        # SBUF tmp, then DMA-shift rows 0..63 -> 64..127
                        tmp = toddp.tile([64, S], BF16, tag="todd")
                        nc.vector.tensor_mul(tmp[:], avc[0:64, :], rb_t[:])
                        nc.sync.dma_start(outT[64:128, j, :], tmp[:])
                del pend[f"avc{j}"], pend[f"rec{j}"]

            # ---------- projection partials (tail PE filler) ----------
            # During pairs 5-7 the qk/v quanta are exhausted; to keep the
            # PE stream dense, emit partial projection sums over the
            # finished outT slices (ko 0..2 + bias) into resident SBUF
            # tiles, finished after the loop with ko 3..7.
            ypart_sb = {}
            KPART = 3

            def proj_partial(st, nE):
                def go():
                    # nE=0 weights come from the wv resident (re-loaded with
                    # wpT half 0 after the v quanta retire); nE=1 from the
                    # xT alias (re-loaded with full wpT during pair 6)
                    ps = psS.tile([P, 512], F32, tag="ps",
                                  name=f"pp{st}_{nE}")
                    for ko in range(KPART):
                        w = (wv_res["wp0"][:, ko, :] if nE == 0
                             else xT[:, ko, 512:1024])
                        nc.tensor.matmul(
                            ps[:], outT[:, ko, st * P:(st + 1) * P], w,
                            start=(ko == 0), stop=False)
                    nc.tensor.matmul(
                        ps[:], ones1x128[:],
                        beff_sb[:, nE * 512:(nE + 1) * 512],
                        start=False, stop=True)
                    yp = ypartp.tile([P, 512], F32, tag="yp",
                                     name=f"yp{st}_{nE}")
                    ypart_sb[(st, nE)] = yp
                    if (st + nE) % 2 == 0:
                        nc.scalar.copy(yp[:], ps[:])
                    else:
                        nc.vector.tensor_copy(yp[:], ps[:])
                return go

            def wp0_dma():
                wt = wvp.tile([P, KO, 512], BF16, tag="wv", name="wp0")
                nc.sync.dma_start(wt[:], wpT_v[:, :, 0:512])
                wv_res["wp0"] = wt

            # ---------- interleaved emission ----------
            # prologue: qk for pairs 0,1 and v half 0; weight DMAs for
            # pairs 2,3 interleaved so they land ~2 pairs ahead of use
            for q in qk_quanta(0):
                q()
            wqk_dma(2)
            for q in qk_quanta(1):
                q()
            for q in v_quanta(0):
                q()
            wv_dma(1)
            vwork = list(v_quanta(1))   # needed from pair 4 on

            for j in range(NPAIRS):
                # qkv work to interleave into this pair's m-steps; the
                # weight DMA for pair j+3 goes first so it lands a full
                # pair ahead of its consuming quanta (emitted at pair j+1)
                work = []
                if j + 3 < NPAIRS:
                    work.append(lambda j=j: wqk_dma(j + 3))
                if j + 2 < NPAIRS:
                    work.extend(qk_quanta(j + 2))
                if j in (1, 2) and vwork:
                    for _ in range(4):
                        work.append(vwork.pop(0))
                if j == 3:
                    # rb_norm(2) lands at m=4, so partials pop at m>=5
                    work.insert(1, wp0_dma)
                    work.extend([proj_partial(0, 0), proj_partial(1, 0)])
                if j == 4:
                    work.extend([proj_partial(2, 0), proj_partial(3, 0),
                                 proj_partial(4, 0)])
                if j == 5:
                    work.extend([proj_partial(5, 0), proj_partial(6, 0),
                                 proj_partial(7, 0)])
                if j == 6:
                    # xT is dead (all QKV matmuls emitted); stream wpT into
                    # its SBUF space, interleaved with nE=1 partials
                    for ko in range(KO):
                        work.append(lambda ko=ko: nc.sync.dma_start(
                            xT[:, ko, :], wpT_v[:, ko, :]))
                        if ko >= 4:
                            work.append(proj_partial(ko - 4, 1))
                if j == 7:
                    work.extend([proj_partial(st, 1) for st in range(4, ST)])
                for m in range(ST):
                    scores_exp(j, m)
                    if m == 4 and j > 0:
                        rb_norm(j - 1)
                    if m == 0:
                        pend[f"ps{j}"] = [
                            [psAV.tile([65, 512], F32, tag="av",
                                       name=f"av{j}_{hb}_{n}")
                             for n in range(2)] for hb in range(2)]
                    if m >= 2:
                        av_m(j, m - 2)
                    if m == 6:
                        evict_n0(j)
                    if m % 2 == 1 and work:
                        # drain evenly over the remaining odd slots so no
                        # burst of quanta piles up at the pair boundary
                        slots_left = (ST - m + 1) // 2
                        npop = -(-len(work) // slots_left)
                        for _ in range(npop):
                            if work:
                                work.pop(0)()
                av_m(j, ST - 2)
                while work:
                    work.pop(0)()
                av_m(j, ST - 1)
                evict_recip(j)
            rb_norm(NPAIRS - 1)

            # ---------- output projection: finish ko 3..7 ----------
            # The SBUF-resident partial is added (DVE, in place) to the
            # remaining-ko result and DMA'd out.
            for st in range(ST):
                for nE in range(2):
                    # alternate psum pools (psAV is dead now) so up to 8
                    # groups' ko3..6 matmuls pipeline while rb_norm(7)
                    # still gates every group's ko7
                    pool = psS if (st + nE) % 2 == 0 else psAV
                    ps = pool.tile([P, 512], F32,
                                   tag="ps" if pool is psS else "av",
                                   name=f"yps{st}_{nE}")
                    for ko in range(KPART, KO):
                        nc.tensor.matmul(
                            ps[:], outT[:, ko, st * P:(st + 1) * P],
                            xT[:, ko, nE * 512:(nE + 1) * 512],
                            start=(ko == KPART), stop=(ko == KO - 1))
                    yp = ypart_sb[(st, nE)]
                    nc.vector.tensor_add(yp[:], yp[:], ps[:])
                    nc.sync.dma_start(
                        y_d[st * P:(st + 1) * P, nE * 512:(nE + 1) * 512],
                        yp[:])

    nc.compile()
    return nc


def kernel(x, w_attn, b_attn, w_proj, b_proj):
    import ml_dtypes
    import concourse.bass_utils as bass_utils

    if "nc" not in _CACHE:
        _CACHE["nc"] = _build()
    nc = _CACHE["nc"]

    BF = ml_dtypes.bfloat16
    x = np.asarray(x, dtype=np.float32)
    w_attn = np.asarray(w_attn, dtype=np.float32)
    b_attn = np.asarray(b_attn, dtype=np.float32)
    w_proj = np.asarray(w_proj, dtype=np.float32)
    b_proj = np.asarray(b_proj, dtype=np.float32)

    xT = np.ascontiguousarray(
        np.transpose(x, (0, 2, 1)).astype(BF))                   # [B, D, S]
    # [D, 2D] -> [p, m, ko, e] so each m-tile is contiguous per partition
    wqkT = np.ascontiguousarray(
        w_attn[:2 * D].T.reshape(KO, P, MT, P).transpose(1, 2, 0, 3)
        .astype(BF))
    wvT = np.ascontiguousarray(w_attn[2 * D:].T.astype(BF))      # [D, D]
    wpT = np.ascontiguousarray(w_proj.T.astype(BF))              # [D, D]
    bqk = np.ascontiguousarray(b_attn[:2 * D])
    bv = b_attn[2 * D:]
    beff = (b_proj.astype(np.float64)
            + w_proj.astype(np.float64) @ bv.astype(np.float64)
            ).astype(np.float32)
    umask = np.triu(np.ones((P, P), dtype=np.float32))           # f >= p
    in_maps = [
        dict(xT=xT[b], wqkT=wqkT, wvT=wvT, wpT=wpT, bqk=bqk, beff=beff,
             umask=umask)
        for b in range(B)
    ]
    res = bass_utils.run_bass_kernel_spmd(
        nc, in_maps, core_ids=list(range(NCORES)), trace=TRACE)
    if TRACE:
        _CACHE["exec_time_ns"] = res.exec_time_ns
        _CACHE["trace"] = res.instructions_and_trace
    return np.stack([res.results[b]["y"] for b in range(B)], axis=0)


# revision 71
# speedup vs baseline: 1.1085x; 1.1085x over previous
"""Causal self-attention on 8 TRN2 NeuronCores, batch-data-parallel (one batch
element per core).

Layout strategy (per core, S=1024, D=1024, H=16, hd=64):
  - Host pre-transposes x -> xT [D,S] and all weights -> [in_dim, out_dim],
    all converted to bf16 (same 1 col/cycle PE rate as fp32r but much lower
    PE power draw -- sustained fp32r trips the ~200us thermal firmware loop
    which gates the PE clock to 4/8 for the rest of the run).
  - qk projection produces q,k transposed ([e,s]) per head-pair: lhsT = wqkT
    tiles, rhs = xT.  Head h lives at partitions 64*(h%2)..+64.
  - v natural [s,e]: lhsT = xT tiles, rhs = resident wv half tiles; stored
    interleaved with a ones column per head (65 cols/head) so the AV matmul's
    PSUM row 64 is the softmax denominator (rowsum of unnormalized attn).
  - scoresT [sk,sq] per head-pair via K=64 matmuls; exp on ACT (scale=1/8
    folded in); causal diag masked by multiplicative upper-tri mask (DVE);
    fully-masked tiles never computed; AV rhs ranges narrowed to the
    causally-written columns (no zero-fills needed).
  - AV: outT'[hd+1, sq] accumulated m-major; normalization via
    approx-reciprocal of the den row (fp32) + PE rank-1 broadcast + DVE
    multiply (writes bf16 outT).
  - proj: y[s,e] with lhsT = outT tiles, rhs = wpT (DMA'd over the dead xT
    SBUF space during pairs 6-7) + rank-1 bias term (beff = b_proj +
    W_proj @ b_v; b_v folds exactly through softmax rowsum).  ko 0..2 are
    computed as partial sums interleaved into pairs 3-7 (tail PE filler),
    ko 3..7 finish after the loop with a DVE add of the DRAM partial.
  - QKV matmul quanta are interleaved into the attention pair loop; weight
    DMAs are hoisted 1+ pairs ahead of their consuming matmuls.
"""

import numpy as np

B, S, D, H = 8, 1024, 1024, 16
HD = D // H          # 64
P = 128
NCORES = 8
KO = D // P          # 8 contraction tiles over d
MT = (2 * D) // P    # 16 m-tiles for q,k
ST = S // P          # 8 s-tiles
NPAIRS = H // 2      # 8 head pairs

_CACHE = {}
TRACE = False        # set by test harness to collect an NTFF profile


def _score_chunks(w):
    # split w into <=512-wide pieces (PSUM bank limit); bf16 runs at full
    # rate at any width
    table = {1024: [512, 512], 896: [512, 384], 768: [512, 256],
             640: [384, 256], 512: [512], 384: [384], 256: [256], 128: [128]}
    return table[w]


def _build():
    import concourse.tile as tile
    from concourse import bacc, mybir

    F32R = mybir.dt.float32r
    F32 = mybir.dt.float32
    BF16 = mybir.dt.bfloat16
    AF = mybir.ActivationFunctionType

    nc = bacc.Bacc("TRN2", target_bir_lowering=False, debug=False,
                   num_devices=NCORES)
    xT_d = nc.dram_tensor("xT", [D, S], BF16, kind="ExternalInput").ap()
    # wqkT host-laid-out as [p, m, ko, e] so each m-tile's DMA reads
    # contiguous lines per partition
    wqkT_d = nc.dram_tensor("wqkT", [P, MT, KO, P], BF16,
                            kind="ExternalInput").ap()
    wvT_d = nc.dram_tensor("wvT", [D, D], BF16, kind="ExternalInput").ap()
    wpT_d = nc.dram_tensor("wpT", [D, D], BF16, kind="ExternalInput").ap()
    bqk_d = nc.dram_tensor("bqk", [2 * D], F32, kind="ExternalInput").ap()
    beff_d = nc.dram_tensor("beff", [D], F32R, kind="ExternalInput").ap()
    umask_d = nc.dram_tensor("umask", [P, P], F32, kind="ExternalInput").ap()
    y_d = nc.dram_tensor("y", [S, D], F32, kind="ExternalOutput").ap()

    wvT_v = wvT_d.rearrange("(ko p) e -> p ko e", p=P)
    wpT_v = wpT_d.rearrange("(ko p) e -> p ko e", p=P)
    xT_v = xT_d.rearrange("(ko p) s -> p ko s", p=P)

    with tile.TileContext(nc) as tc:
        with (
            tc.tile_pool(name="bigio", bufs=1) as bigio,
            tc.tile_pool(name="qkp", bufs=3) as qkp,
            tc.tile_pool(name="vp", bufs=1) as vpool,
            tc.tile_pool(name="wqk", bufs=4) as wqkp,
            tc.tile_pool(name="wvp", bufs=1) as wvp,
            tc.tile_pool(name="attn", bufs=6) as attnp,
            tc.tile_pool(name="rt", bufs=2) as rtp,
            tc.tile_pool(name="rb", bufs=1) as rbp,
            tc.tile_pool(name="todd", bufs=1) as toddp,
            tc.tile_pool(name="ypart", bufs=16) as ypartp,
            tc.tile_pool(name="avsb", bufs=2) as avsbp,
            tc.tile_pool(name="cst", bufs=1) as cst,
            tc.tile_pool(name="psS", bufs=4, space="PSUM") as psS,
            tc.tile_pool(name="psAV", bufs=2, space="PSUM") as psAV,
        ):
            # ---------- constants ----------
            umask = cst.tile([P, P], F32)
            bqk_sb = cst.tile([P, MT], F32)
            beff_sb = cst.tile([1, D], F32R)
            onecol = cst.tile([P, 1], F32)
            nc.vector.memset(onecol[:], 1.0)
            ones1x128 = cst.tile([1, P], F32R)
            nc.vector.tensor_copy(
                ones1x128[:], onecol[0:1, :].broadcast_to([1, P]))
            ones65r = cst.tile([65, 64], F32R)
            nc.vector.memset(ones65r[64:65, :].bitcast(F32), 1.0)

            # ---------- big SBUF residents ----------
            # xT is overwritten with wpT during pairs 6-7 (QKV matmuls are
            # done reading it by then); proj reads it as the wpT resident.
            xT = bigio.tile([P, KO, S], BF16, tag="xT")
            outT = bigio.tile([P, KO, S], BF16, tag="outT")
            v_sb = vpool.tile([P, ST, H * (HD + 1)], BF16)
            v_hview = v_sb[:].rearrange("p st (h c) -> p st h c", c=HD + 1)

            # ---------- input DMA schedule ----------
            wqk_tiles = {}

            def wqk_dma(j):
                for part in (0, 1):
                    m = j if part == 0 else NPAIRS + j
                    wt = wqkp.tile([P, KO, P], BF16, tag="wqk",
                                   name=f"wqk{m}")
                    nc.sync.dma_start(wt[:], wqkT_d[:, m, :, :])
                    wqk_tiles[m] = wt

            wv_res = {}

            def wv_dma(nE):
                wt = wvp.tile([P, KO, 512], BF16, tag="wv", name=f"wv{nE}")
                nc.sync.dma_start(wt[:],
                                  wvT_v[:, :, nE * 512:(nE + 1) * 512])
                wv_res[nE] = wt

            # critical-path DMAs first: the sync queue issues each dma_start
            # serially (~0.8us apiece), so the first quantum's inputs go
            # ahead of everything else
            # pair-0 q weights, then the xT wave; k weights (m=8) aren't
            # read until ~3.5us of q matmuls have run
            wt0 = wqkp.tile([P, KO, P], BF16, tag="wqk", name="wqk0")
            nc.sync.dma_start(wt0[:], wqkT_d[:, 0, :, :])
            wqk_tiles[0] = wt0
            nc.sync.dma_start(xT[:, 0, :], xT_v[:, 0, :])
            nc.sync.dma_start(xT[:, 1, :], xT_v[:, 1, :])
            wt8 = wqkp.tile([P, KO, P], BF16, tag="wqk", name="wqk8")
            nc.sync.dma_start(wt8[:], wqkT_d[:, NPAIRS, :, :])
            wqk_tiles[NPAIRS] = wt8
            nc.sync.dma_start(xT[:, 2, :], xT_v[:, 2, :])
            wqk_dma(1)
            for ko in range(3, KO):
                nc.sync.dma_start(xT[:, ko, :], xT_v[:, ko, :])
            wv_dma(0)
            nc.sync.dma_start(umask[:], umask_d)
            nc.sync.dma_start(bqk_sb[:], bqk_d.rearrange("(m p) -> p m", p=P))
            nc.sync.dma_start(beff_sb[:], beff_d[None, :])
            nc.vector.tensor_copy(
                v_hview[:, :, :, HD:HD + 1],
                onecol[:, None, None, :].broadcast_to([P, ST, H, 1]))

            qk_tiles = {}    # j -> [128, 2, S] tile (0=q, 1=k)

            # ---------- QKV work quanta (emitted interleaved) ----------
            def qk_quanta(j):
                # 4 closures; each computes one (part, nn) psum group.
                # Weights were DMA'd earlier via wqk_dma(j).
                t = qkp.tile([P, 2, S], BF16, tag="qkt", name=f"qk{j}")
                qk_tiles[j] = t

                def quantum(part, nn):    # part 0=q (m-tile j), 1=k (8+j)
                    def go():
                        m = j if part == 0 else NPAIRS + j
                        wt = wqk_tiles[m]
                        ps = psS.tile([P, 512], F32, tag="ps", name=f"qkps{m}")
                        for ko in range(KO):
                            nc.tensor.matmul(
                                ps[:], wt[:, ko, :],
                                xT[:, ko, nn * 512:(nn + 1) * 512],
                                start=(ko == 0), stop=(ko == KO - 1))
                        nc.vector.tensor_scalar_add(
                            t[:, part, nn * 512:(nn + 1) * 512], ps[:],
                            bqk_sb[:, m:m + 1])
                    return go
                return [quantum(0, 0), quantum(0, 1),
                        quantum(1, 0), quantum(1, 1)]

            def v_quanta(nE):
                # v half nE: e_v cols 512*nE.. (heads 8nE..8nE+7), 8 quanta
                # of 1 s-tile each, reading the resident wv half tile
                def quantum(st):
                    def go():
                        wt = wv_res[nE]
                        ps = psS.tile([P, 512], F32, tag="ps",
                                      name=f"vps{nE}_{st}")
                        for ko in range(KO):
                            nc.tensor.matmul(
                                ps[:], xT[:, ko, st * P:(st + 1) * P],
                                wt[:, ko, :], start=(ko == 0),
                                stop=(ko == KO - 1))
                        nc.vector.tensor_copy(
                            v_hview[:, st, 8 * nE:8 * (nE + 1), 0:HD],
                            ps[:].rearrange("p (h c) -> p h c", c=HD))
                    return go
                return [quantum(st) for st in range(ST)]

            # ---------- attention ----------
            pend = {}

            def scores_exp(j, m):
                qk_t = qk_tiles[j]
                w = S - m * P
                for hb, base in ((0, 0), (1, 64)):   # head 2j+hb
                    at = attnp.tile([P, S], BF16, tag="at",
                                    name=f"at{j}_{hb}_{m}")
                    pend[(j, hb, m)] = at
                    off = m * P
                    for cw in _score_chunks(w):
                        ps = psS.tile([P, 512], F32, tag="ps",
                                      name=f"sps{j}_{hb}_{m}")
                        nc.tensor.matmul(
                            ps[:, 0:cw],
                            qk_t[base:base + 64, 1, m * P:(m + 1) * P],
                            qk_t[base:base + 64, 0, off:off + cw],
                            start=True, stop=True)
                        nc.scalar.activation(
                            at[:, off:off + cw], ps[:, 0:cw], AF.Exp,
                            scale=0.125)
                        off += cw
                    nc.vector.tensor_mul(
                        at[:, m * P:(m + 1) * P], at[:, m * P:(m + 1) * P],
                        umask[:])

            def av_m(j, m):
                # rhs narrowed to the causally-written at columns, so the
                # at tiles need no zero-fill of the masked prefix
                st8 = pend[f"ps{j}"]
                for hb in (0, 1):
                    h = 2 * j + hb
                    at = pend[(j, hb, m)]
                    for n in range((0 if m <= 3 else 1), 2):
                        lo = max(n * 512, m * P)
                        nc.tensor.matmul(
                            st8[hb][:, lo:(n + 1) * 512],
                            v_sb[:, m, h * (HD + 1):(h + 1) * (HD + 1)],
                            at[:, lo:(n + 1) * 512],
                            start=(m == 0), stop=(m == 4 * n + 3))

            def evict_recip(j):
                # move the [65, S] AV accumulators out of PSUM so the next
                # pair's AV matmuls get the PSUM slots immediately, then take
                # the reciprocal of the den row (approx_fast: ~4e-6 rel;
                # den >= exp(0) > 0 so no edge cases)
                avcs, recs = [], []
                for hb in (0, 1):
                    avc = avsbp.tile([65, S], F32R, tag="avc",
                                     name=f"avc{j}_{hb}")
                    # split each eviction's halves across ACT and DVE so
                    # they run in parallel (they gate the next pair's AV)
                    src = pend[f"ps{j}"][hb]
                    eng = (nc.scalar.copy, nc.vector.tensor_copy)
                    for c in range(2):
                        eng[(hb + c) % 2](
                            avc[:, c * 512:(c + 1) * 512],
                            src[:, c * 512:(c + 1) * 512])
                    avcs.append(avc)
                    rt = rtp.tile([65, S], F32R, tag="rt")
                    # custom-DVE op misbehaves on single-partition APs on HW:
                    # run it over all 65 rows (lanes are parallel) and consume
                    # only the den row (64); other lanes are never read
                    for c in range(4):
                        rt32 = rtp.tile([65, 256], F32, tag="rt32", bufs=1)
                        nc.vector.reciprocal_approx_fast(
                            rt32[:],
                            avc[:, c * 256:(c + 1) * 256].bitcast(F32))
                        nc.vector.tensor_copy(
                            rt[64:65, c * 256:(c + 1) * 256], rt32[64:65, :])
                    recs.append(rt)
                pend[f"avc{j}"] = avcs
                pend[f"rec{j}"] = recs
                del pend[f"ps{j}"]

            def rb_norm(j):
                for hb in (0, 1):
                    rt = pend[f"rec{j}"][hb]
                    rb_t = rbp.tile([64, S], F32R, tag="rb")
                    for c in range(2):
                        rps = psS.tile([P, 512], F32, tag="ps",
                                       name=f"rbps{j}_{hb}_{c}")
                        nc.tensor.matmul(
                            rps[0:64, :], ones65r[64:65, :],
                            rt[64:65, c * 512:(c + 1) * 512],
                            start=True, stop=True)
                        nc.scalar.copy(
                            rb_t[:, c * 512:(c + 1) * 512], rps[0:64, :])
                    avc = pend[f"avc{j}"][hb]
                    if hb == 0:
                        nc.vector.tensor_mul(
                            outT[0:64, j, :], avc[0:64, :], rb_t[:])
                    else:
                        # lanes cannot shift partitions: multiply to an
                        # SBUF tmp, then DMA-shift rows 0..63 -> 64..127
                        tmp = toddp.tile([64, S], BF16, tag="todd")
                        nc.vector.tensor_mul(tmp[:], avc[0:64, :], rb_t[:])
                        nc.sync.dma_start(outT[64:128, j, :], tmp[:])
                del pend[f"avc{j}"], pend[f"rec{j}"]

            # ---------- projection partials (tail PE filler) ----------
            # During pairs 5-7 the qk/v quanta are exhausted; to keep the
            # PE stream dense, emit partial projection sums over the
            # finished outT slices (ko 0..2 + bias) into resident SBUF
            # tiles, finished after the loop with ko 3..7.
            ypart_sb = {}
            KPART = 3

            def proj_partial(st, nE):
                def go():
                    # nE=0 weights come from the wv resident (re-loaded with
                    # wpT half 0 after the v quanta retire); nE=1 from the
                    # xT alias (re-loaded with full wpT during pair 6)
                    ps = psS.tile([P, 512], F32, tag="ps",
                                  name=f"pp{st}_{nE}")
                    for ko in range(KPART):
                        w = (wv_res["wp0"][:, ko, :] if nE == 0
                             else xT[:, ko, 512:1024])
                        nc.tensor.matmul(
                            ps[:], outT[:, ko, st * P:(st + 1) * P], w,
                            start=(ko == 0), stop=False)
                    nc.tensor.matmul(
                        ps[:], ones1x128[:],
                        beff_sb[:, nE * 512:(nE + 1) * 512],
                        start=False, stop=True)
                    yp = ypartp.tile([P, 512], F32, tag="yp",
                                     name=f"yp{st}_{nE}")
                    ypart_sb[(st, nE)] = yp
                    if (st + nE) % 2 == 0:
                        nc.scalar.copy(yp[:], ps[:])
                    else:
                        nc.vector.tensor_copy(yp[:], ps[:])
                return go

            def wp0_dma():
                wt = wvp.tile([P, KO, 512], BF16, tag="wv", name="wp0")
                nc.sync.dma_start(wt[:], wpT_v[:, :, 0:512])
                wv_res["wp0"] = wt

            # ---------- interleaved emission ----------
            # prologue: qk for pairs 0,1 and v half 0; weight DMAs for
            # pairs 2,3 interleaved so they land ~2 pairs ahead of use
            for q in qk_quanta(0):
                q()
            wqk_dma(2)
            for q in qk_quanta(1):
                q()
            for q in v_quanta(0):
                q()
            wv_dma(1)
            vwork = list(v_quanta(1))   # needed from pair 4 on

            for j in range(NPAIRS):
                # qkv work to interleave into this pair's m-steps; the
                # weight DMA for pair j+3 goes first so it lands a full
                # pair ahead of its consuming quanta (emitted at pair j+1)
                work = []
                if j + 3 < NPAIRS:
                    work.append(lambda j=j: wqk_dma(j + 3))
                if j + 2 < NPAIRS:
                    work.extend(qk_quanta(j + 2))
                if j in (1, 2) and vwork:
                    for _ in range(4):
                        work.append(vwork.pop(0))
                if j == 3:
                    # rb_norm(2) lands at m=4, so partials pop at m>=5
                    work.insert(1, wp0_dma)
                    work.extend([proj_partial(0, 0), proj_partial(1, 0)])
                if j == 4:
                    work.extend([proj_partial(2, 0), proj_partial(3, 0),
                                 proj_partial(4, 0)])
                if j == 5:
                    work.extend([proj_partial(5, 0), proj_partial(6, 0),
                                 proj_partial(7, 0)])
                if j == 6:
                    # xT is dead (all QKV matmuls emitted); stream wpT into
                    # its SBUF space, interleaved with nE=1 partials
                    for ko in range(KO):
                        work.append(lambda ko=ko: nc.sync.dma_start(
                            xT[:, ko, :], wpT_v[:, ko, :]))
                        if ko >= 4:
                            work.append(proj_partial(ko - 4, 1))
                if j == 7:
                    work.extend([proj_partial(st, 1) for st in range(4, ST)])
                for m in range(ST):
                    scores_exp(j, m)
                    if m == 4 and j > 0:
                        rb_norm(j - 1)
                    if m == 0:
                        pend[f"ps{j}"] = [
                            psAV.tile([65, S], F32, tag="av",
                                      name=f"av{j}_{hb}") for hb in range(2)]
                    if m >= 2:
                        av_m(j, m - 2)
                    if m % 2 == 1 and work:
                        # drain evenly over the remaining odd slots so no
                        # burst of quanta piles up at the pair boundary
                        slots_left = (ST - m + 1) // 2
                        npop = -(-len(work) // slots_left)
                        for _ in range(npop):
                            if work:
                                work.pop(0)()
                av_m(j, ST - 2)
                while work:
                    work.pop(0)()
                av_m(j, ST - 1)
                evict_recip(j)
            rb_norm(NPAIRS - 1)

            # ---------- output projection: finish ko 3..7 ----------
            # The SBUF-resident partial is added (DVE, in place) to the
            # remaining-ko result and DMA'd out.
            for st in range(ST):
                for nE in range(2):
                    # alternate psum pools (psAV is dead now) so up to 8
                    # groups' ko3..6 matmuls pipeline while rb_norm(7)
                    # still gates every group's ko7
                    pool = psS if (st + nE) % 2 == 0 else psAV
                    ps = pool.tile([P, 512], F32,
                                   tag="ps" if pool is psS else "av",
                                   name=f"yps{st}_{nE}")
                    for ko in range(KPART, KO):
                        nc.tensor.matmul(
                            ps[:], outT[:, ko, st * P:(st + 1) * P],
                            xT[:, ko, nE * 512:(nE + 1) * 512],
                            start=(ko == KPART), stop=(ko == KO - 1))
                    yp = ypart_sb[(st, nE)]
                    nc.vector.tensor_add(yp[:], yp[:], ps[:])
                    nc.sync.dma_start(
                        y_d[st * P:(st + 1) * P, nE * 512:(nE + 1) * 512],
                        yp[:])

    nc.compile()
    return nc


def kernel(x, w_attn, b_attn, w_proj, b_proj):
    import ml_dtypes
    import concourse.bass_utils as bass_utils

    if "nc" not in _CACHE:
        _CACHE["nc"] = _build()
    nc = _CACHE["nc"]

    BF = ml_dtypes.bfloat16
    x = np.asarray(x, dtype=np.float32)
    w_attn = np.asarray(w_attn, dtype=np.float32)
    b_attn = np.asarray(b_attn, dtype=np.float32)
    w_proj = np.asarray(w_proj, dtype=np.float32)
    b_proj = np.asarray(b_proj, dtype=np.float32)

    xT = np.ascontiguousarray(
        np.transpose(x, (0, 2, 1)).astype(BF))                   # [B, D, S]
    # [D, 2D] -> [p, m, ko, e] so each m-tile is contiguous per partition
    wqkT = np.ascontiguousarray(
        w_attn[:2 * D].T.reshape(KO, P, MT, P).transpose(1, 2, 0, 3)
        .astype(BF))
    wvT = np.ascontiguousarray(w_attn[2 * D:].T.astype(BF))      # [D, D]
    wpT = np.ascontiguousarray(w_proj.T.astype(BF))              # [D, D]
    bqk = np.ascontiguousarray(b_attn[:2 * D])
    bv = b_attn[2 * D:]
    beff = (b_proj.astype(np.float64)
            + w_proj.astype(np.float64) @ bv.astype(np.float64)
            ).astype(np.float32)
    umask = np.triu(np.ones((P, P), dtype=np.float32))           # f >= p
    in_maps = [
        dict(xT=xT[b], wqkT=wqkT, wvT=wvT, wpT=wpT, bqk=bqk, beff=beff,
             umask=umask)
        for b in range(B)
    ]
    res = bass_utils.run_bass_kernel_spmd(
        nc, in_maps, core_ids=list(range(NCORES)), trace=TRACE)
    if TRACE:
        _CACHE["exec_time_ns"] = res.exec_time_ns
        _CACHE["trace"] = res.instructions_and_trace
    return np.stack([res.results[b]["y"] for b in range(B)], axis=0)


# revision 73
# speedup vs baseline: 1.1131x; 1.0042x over previous
"""Causal self-attention on 8 TRN2 NeuronCores, batch-data-parallel (one batch
element per core).

Layout strategy (per core, S=1024, D=1024, H=16, hd=64):
  - Host pre-transposes x -> xT [D,S] and all weights -> [in_dim, out_dim],
    all converted to bf16 (same 1 col/cycle PE rate as fp32r but much lower
    PE power draw -- sustained fp32r trips the ~200us thermal firmware loop
    which gates the PE clock to 4/8 for the rest of the run).
  - qk projection produces q,k transposed ([e,s]) per head-pair: lhsT = wqkT
    tiles, rhs = xT.  Head h lives at partitions 64*(h%2)..+64.
  - v natural [s,e]: lhsT = xT tiles, rhs = resident wv half tiles; stored
    interleaved with a ones column per head (65 cols/head) so the AV matmul's
    PSUM row 64 is the softmax denominator (rowsum of unnormalized attn).
  - scoresT [sk,sq] per head-pair via K=64 matmuls; exp on ACT (scale=1/8
    folded in); causal diag masked by multiplicative upper-tri mask (DVE);
    fully-masked tiles never computed; AV rhs ranges narrowed to the
    causally-written columns (no zero-fills needed).
  - AV: outT'[hd+1, sq] accumulated m-major; normalization via
    approx-reciprocal of the den row (fp32) + PE rank-1 broadcast + DVE
    multiply (writes bf16 outT).
  - proj: y[s,e] with lhsT = outT tiles, rhs = wpT (DMA'd over the dead xT
    SBUF space during pairs 6-7) + rank-1 bias term (beff = b_proj +
    W_proj @ b_v; b_v folds exactly through softmax rowsum).  ko 0..2 are
    computed as partial sums interleaved into pairs 3-7 (tail PE filler),
    ko 3..7 finish after the loop with a DVE add of the DRAM partial.
  - QKV matmul quanta are interleaved into the attention pair loop; weight
    DMAs are hoisted 1+ pairs ahead of their consuming matmuls.
"""

import numpy as np

B, S, D, H = 8, 1024, 1024, 16
HD = D // H          # 64
P = 128
NCORES = 8
KO = D // P          # 8 contraction tiles over d
MT = (2 * D) // P    # 16 m-tiles for q,k
ST = S // P          # 8 s-tiles
NPAIRS = H // 2      # 8 head pairs

_CACHE = {}
TRACE = False        # set by test harness to collect an NTFF profile


def _score_chunks(w):
    # split w into <=512-wide pieces (PSUM bank limit); bf16 runs at full
    # rate at any width
    table = {1024: [512, 512], 896: [512, 384], 768: [512, 256],
             640: [384, 256], 512: [512], 384: [384], 256: [256], 128: [128]}
    return table[w]


def _build():
    import concourse.tile as tile
    from concourse import bacc, mybir

    F32R = mybir.dt.float32r
    F32 = mybir.dt.float32
    BF16 = mybir.dt.bfloat16
    AF = mybir.ActivationFunctionType

    nc = bacc.Bacc("TRN2", target_bir_lowering=False, debug=False,
                   num_devices=NCORES)
    xT_d = nc.dram_tensor("xT", [D, S], BF16, kind="ExternalInput").ap()
    # wqkT host-laid-out as [p, m, ko, e] so each m-tile's DMA reads
    # contiguous lines per partition
    wqkT_d = nc.dram_tensor("wqkT", [P, MT, KO, P], BF16,
                            kind="ExternalInput").ap()
    wvT_d = nc.dram_tensor("wvT", [D, D], BF16, kind="ExternalInput").ap()
    wpT_d = nc.dram_tensor("wpT", [D, D], BF16, kind="ExternalInput").ap()
    bqk_d = nc.dram_tensor("bqk", [2 * D], F32, kind="ExternalInput").ap()
    beff_d = nc.dram_tensor("beff", [D], F32R, kind="ExternalInput").ap()
    umask_d = nc.dram_tensor("umask", [P, P], F32, kind="ExternalInput").ap()
    y_d = nc.dram_tensor("y", [S, D], F32, kind="ExternalOutput").ap()

    wvT_v = wvT_d.rearrange("(ko p) e -> p ko e", p=P)
    wpT_v = wpT_d.rearrange("(ko p) e -> p ko e", p=P)
    xT_v = xT_d.rearrange("(ko p) s -> p ko s", p=P)

    with tile.TileContext(nc) as tc:
        with (
            tc.tile_pool(name="bigio", bufs=1) as bigio,
            tc.tile_pool(name="qkp", bufs=3) as qkp,
            tc.tile_pool(name="vp", bufs=1) as vpool,
            tc.tile_pool(name="wqk", bufs=6) as wqkp,
            tc.tile_pool(name="wvp", bufs=1) as wvp,
            tc.tile_pool(name="attn", bufs=6) as attnp,
            tc.tile_pool(name="rt", bufs=2) as rtp,
            tc.tile_pool(name="rb", bufs=1) as rbp,
            tc.tile_pool(name="todd", bufs=1) as toddp,
            tc.tile_pool(name="ypart", bufs=16) as ypartp,
            tc.tile_pool(name="avsb", bufs=2) as avsbp,
            tc.tile_pool(name="cst", bufs=1) as cst,
            tc.tile_pool(name="psS", bufs=4, space="PSUM") as psS,
            tc.tile_pool(name="psAV", bufs=2, space="PSUM") as psAV,
        ):
            # ---------- constants ----------
            umask = cst.tile([P, P], F32)
            bqk_sb = cst.tile([P, MT], F32)
            beff_sb = cst.tile([1, D], F32R)
            onecol = cst.tile([P, 1], F32)
            nc.vector.memset(onecol[:], 1.0)
            ones1x128 = cst.tile([1, P], F32R)
            nc.vector.tensor_copy(
                ones1x128[:], onecol[0:1, :].broadcast_to([1, P]))
            ones65r = cst.tile([65, 64], F32R)
            nc.vector.memset(ones65r[64:65, :].bitcast(F32), 1.0)

            # ---------- big SBUF residents ----------
            # xT is overwritten with wpT during pairs 6-7 (QKV matmuls are
            # done reading it by then); proj reads it as the wpT resident.
            xT = bigio.tile([P, KO, S], BF16, tag="xT")
            outT = bigio.tile([P, KO, S], BF16, tag="outT")
            v_sb = vpool.tile([P, ST, H * (HD + 1)], BF16)
            v_hview = v_sb[:].rearrange("p st (h c) -> p st h c", c=HD + 1)

            # ---------- input DMA schedule ----------
            wqk_tiles = {}

            def wqk_dma(j):
                for part in (0, 1):
                    m = j if part == 0 else NPAIRS + j
                    wt = wqkp.tile([P, KO, P], BF16, tag="wqk",
                                   name=f"wqk{m}")
                    nc.sync.dma_start(wt[:], wqkT_d[:, m, :, :])
                    wqk_tiles[m] = wt

            wv_res = {}

            def wv_dma(nE):
                wt = wvp.tile([P, KO, 512], BF16, tag="wv", name=f"wv{nE}")
                nc.sync.dma_start(wt[:],
                                  wvT_v[:, :, nE * 512:(nE + 1) * 512])
                wv_res[nE] = wt

            # critical-path DMAs first: the sync queue issues each dma_start
            # serially (~0.8us apiece), so the first quantum's inputs go
            # ahead of everything else
            # pair-0 q weights, then the xT wave; k weights (m=8) aren't
            # read until ~3.5us of q matmuls have run
            wt0 = wqkp.tile([P, KO, P], BF16, tag="wqk", name="wqk0")
            nc.sync.dma_start(wt0[:], wqkT_d[:, 0, :, :])
            wqk_tiles[0] = wt0
            nc.sync.dma_start(xT[:, 0, :], xT_v[:, 0, :])
            nc.sync.dma_start(xT[:, 1, :], xT_v[:, 1, :])
            wt8 = wqkp.tile([P, KO, P], BF16, tag="wqk", name="wqk8")
            nc.sync.dma_start(wt8[:], wqkT_d[:, NPAIRS, :, :])
            wqk_tiles[NPAIRS] = wt8
            nc.sync.dma_start(xT[:, 2, :], xT_v[:, 2, :])
            wqk_dma(1)
            for ko in range(3, KO):
                nc.sync.dma_start(xT[:, ko, :], xT_v[:, ko, :])
            wv_dma(0)
            nc.sync.dma_start(umask[:], umask_d)
            nc.sync.dma_start(bqk_sb[:], bqk_d.rearrange("(m p) -> p m", p=P))
            nc.sync.dma_start(beff_sb[:], beff_d[None, :])
            nc.vector.tensor_copy(
                v_hview[:, :, :, HD:HD + 1],
                onecol[:, None, None, :].broadcast_to([P, ST, H, 1]))

            qk_tiles = {}    # j -> [128, 2, S] tile (0=q, 1=k)

            # ---------- QKV work quanta (emitted interleaved) ----------
            def qk_quanta(j):
                # 4 closures; each computes one (part, nn) psum group.
                # Weights were DMA'd earlier via wqk_dma(j).
                t = qkp.tile([P, 2, S], BF16, tag="qkt", name=f"qk{j}")
                qk_tiles[j] = t

                def quantum(part, nn):    # part 0=q (m-tile j), 1=k (8+j)
                    def go():
                        m = j if part == 0 else NPAIRS + j
                        wt = wqk_tiles[m]
                        ps = psS.tile([P, 512], F32, tag="ps", name=f"qkps{m}")
                        for ko in range(KO):
                            nc.tensor.matmul(
                                ps[:], wt[:, ko, :],
                                xT[:, ko, nn * 512:(nn + 1) * 512],
                                start=(ko == 0), stop=(ko == KO - 1))
                        nc.vector.tensor_scalar_add(
                            t[:, part, nn * 512:(nn + 1) * 512], ps[:],
                            bqk_sb[:, m:m + 1])
                    return go
                return [quantum(0, 0), quantum(0, 1),
                        quantum(1, 0), quantum(1, 1)]

            def v_quanta(nE):
                # v half nE: e_v cols 512*nE.. (heads 8nE..8nE+7), 8 quanta
                # of 1 s-tile each, reading the resident wv half tile
                def quantum(st):
                    def go():
                        wt = wv_res[nE]
                        ps = psS.tile([P, 512], F32, tag="ps",
                                      name=f"vps{nE}_{st}")
                        for ko in range(KO):
                            nc.tensor.matmul(
                                ps[:], xT[:, ko, st * P:(st + 1) * P],
                                wt[:, ko, :], start=(ko == 0),
                                stop=(ko == KO - 1))
                        nc.vector.tensor_copy(
                            v_hview[:, st, 8 * nE:8 * (nE + 1), 0:HD],
                            ps[:].rearrange("p (h c) -> p h c", c=HD))
                    return go
                return [quantum(st) for st in range(ST)]

            # ---------- attention ----------
            pend = {}

            def scores_exp(j, m):
                qk_t = qk_tiles[j]
                w = S - m * P
                for hb, base in ((0, 0), (1, 64)):   # head 2j+hb
                    at = attnp.tile([P, S], BF16, tag="at",
                                    name=f"at{j}_{hb}_{m}")
                    pend[(j, hb, m)] = at
                    off = m * P
                    for cw in _score_chunks(w):
                        ps = psS.tile([P, 512], F32, tag="ps",
                                      name=f"sps{j}_{hb}_{m}")
                        nc.tensor.matmul(
                            ps[:, 0:cw],
                            qk_t[base:base + 64, 1, m * P:(m + 1) * P],
                            qk_t[base:base + 64, 0, off:off + cw],
                            start=True, stop=True)
                        nc.scalar.activation(
                            at[:, off:off + cw], ps[:, 0:cw], AF.Exp,
                            scale=0.125)
                        off += cw
                    nc.vector.tensor_mul(
                        at[:, m * P:(m + 1) * P], at[:, m * P:(m + 1) * P],
                        umask[:])

            def av_m(j, m):
                # rhs narrowed to the causally-written at columns, so the
                # at tiles need no zero-fill of the masked prefix
                st8 = pend[f"ps{j}"]
                for hb in (0, 1):
                    h = 2 * j + hb
                    at = pend[(j, hb, m)]
                    for n in range((0 if m <= 3 else 1), 2):
                        lo = max(n * 512, m * P)
                        nc.tensor.matmul(
                            st8[hb][:, lo:(n + 1) * 512],
                            v_sb[:, m, h * (HD + 1):(h + 1) * (HD + 1)],
                            at[:, lo:(n + 1) * 512],
                            start=(m == 0), stop=(m == 4 * n + 3))

            def evict_recip(j):
                # move the [65, S] AV accumulators out of PSUM so the next
                # pair's AV matmuls get the PSUM slots immediately, then take
                # the reciprocal of the den row (approx_fast: ~4e-6 rel;
                # den >= exp(0) > 0 so no edge cases)
                avcs, recs = [], []
                for hb in (0, 1):
                    avc = avsbp.tile([65, S], F32R, tag="avc",
                                     name=f"avc{j}_{hb}")
                    # split each eviction's halves across ACT and DVE so
                    # they run in parallel (they gate the next pair's AV)
                    src = pend[f"ps{j}"][hb]
                    eng = (nc.scalar.copy, nc.vector.tensor_copy)
                    for c in range(2):
                        eng[(hb + c) % 2](
                            avc[:, c * 512:(c + 1) * 512],
                            src[:, c * 512:(c + 1) * 512])
                    avcs.append(avc)
                    rt = rtp.tile([65, S], F32R, tag="rt")
                    # custom-DVE op misbehaves on single-partition APs on HW:
                    # run it over all 65 rows (lanes are parallel) and consume
                    # only the den row (64); other lanes are never read
                    for c in range(4):
                        rt32 = rtp.tile([65, 256], F32, tag="rt32", bufs=1)
                        nc.vector.reciprocal_approx_fast(
                            rt32[:],
                            avc[:, c * 256:(c + 1) * 256].bitcast(F32))
                        nc.vector.tensor_copy(
                            rt[64:65, c * 256:(c + 1) * 256], rt32[64:65, :])
                    recs.append(rt)
                pend[f"avc{j}"] = avcs
                pend[f"rec{j}"] = recs
                del pend[f"ps{j}"]

            def rb_norm(j):
                for hb in (0, 1):
                    rt = pend[f"rec{j}"][hb]
                    rb_t = rbp.tile([64, S], F32R, tag="rb")
                    for c in range(2):
                        rps = psS.tile([P, 512], F32, tag="ps",
                                       name=f"rbps{j}_{hb}_{c}")
                        nc.tensor.matmul(
                            rps[0:64, :], ones65r[64:65, :],
                            rt[64:65, c * 512:(c + 1) * 512],
                            start=True, stop=True)
                        nc.scalar.copy(
                            rb_t[:, c * 512:(c + 1) * 512], rps[0:64, :])
                    avc = pend[f"avc{j}"][hb]
                    if hb == 0:
                        nc.vector.tensor_mul(
                            outT[0:64, j, :], avc[0:64, :], rb_t[:])
                    else:
                        # lanes cannot shift partitions: multiply to an
                        # SBUF tmp, then DMA-shift rows 0..63 -> 64..127
                        tmp = toddp.tile([64, S], BF16, tag="todd")
                        nc.vector.tensor_mul(tmp[:], avc[0:64, :], rb_t[:])
                        nc.sync.dma_start(outT[64:128, j, :], tmp[:])
                del pend[f"avc{j}"], pend[f"rec{j}"]

            # ---------- projection partials (tail PE filler) ----------
            # During pairs 5-7 the qk/v quanta are exhausted; to keep the
            # PE stream dense, emit partial projection sums over the
            # finished outT slices (ko 0..2 + bias) into resident SBUF
            # tiles, finished after the loop with ko 3..7.
            ypart_sb = {}
            KPART = 3

            def proj_partial(st, nE):
                def go():
                    # nE=0 weights come from the wv resident (re-loaded with
                    # wpT half 0 after the v quanta retire); nE=1 from the
                    # xT alias (re-loaded with full wpT during pair 6)
                    ps = psS.tile([P, 512], F32, tag="ps",
                                  name=f"pp{st}_{nE}")
                    for ko in range(KPART):
                        w = (wv_res["wp0"][:, ko, :] if nE == 0
                             else xT[:, ko, 512:1024])
                        nc.tensor.matmul(
                            ps[:], outT[:, ko, st * P:(st + 1) * P], w,
                            start=(ko == 0), stop=False)
                    nc.tensor.matmul(
                        ps[:], ones1x128[:],
                        beff_sb[:, nE * 512:(nE + 1) * 512],
                        start=False, stop=True)
                    yp = ypartp.tile([P, 512], F32, tag="yp",
                                     name=f"yp{st}_{nE}")
                    ypart_sb[(st, nE)] = yp
                    if (st + nE) % 2 == 0:
                        nc.scalar.copy(yp[:], ps[:])
                    else:
                        nc.vector.tensor_copy(yp[:], ps[:])
                return go

            def wp0_dma():
                wt = wvp.tile([P, KO, 512], BF16, tag="wv", name="wp0")
                nc.sync.dma_start(wt[:], wpT_v[:, :, 0:512])
                wv_res["wp0"] = wt

            # ---------- interleaved emission ----------
            # prologue: qk for pairs 0,1 and v half 0; weight DMAs for
            # pairs 2,3 interleaved so they land ~2 pairs ahead of use
            for q in qk_quanta(0):
                q()
            wqk_dma(2)
            for q in qk_quanta(1):
                q()
            for q in v_quanta(0):
                q()
            wv_dma(1)
            vwork = list(v_quanta(1))   # needed from pair 4 on

            for j in range(NPAIRS):
                # qkv work to interleave into this pair's m-steps; the
                # weight DMA for pair j+3 goes first so it lands a full
                # pair ahead of its consuming quanta (emitted at pair j+1)
                work = []
                if j + 3 < NPAIRS:
                    work.append(lambda j=j: wqk_dma(j + 3))
                if j + 2 < NPAIRS:
                    work.extend(qk_quanta(j + 2))
                if j in (1, 2) and vwork:
                    for _ in range(4):
                        work.append(vwork.pop(0))
                if j == 3:
                    # rb_norm(2) lands at m=4, so partials pop at m>=5
                    work.insert(1, wp0_dma)
                    work.extend([proj_partial(0, 0), proj_partial(1, 0)])
                if j == 4:
                    work.extend([proj_partial(2, 0), proj_partial(3, 0),
                                 proj_partial(4, 0)])
                if j == 5:
                    work.extend([proj_partial(5, 0), proj_partial(6, 0),
                                 proj_partial(7, 0)])
                if j == 6:
                    # xT is dead (all QKV matmuls emitted); stream wpT into
                    # its SBUF space, interleaved with nE=1 partials
                    for ko in range(KO):
                        work.append(lambda ko=ko: nc.sync.dma_start(
                            xT[:, ko, :], wpT_v[:, ko, :]))
                        if ko >= 4:
                            work.append(proj_partial(ko - 4, 1))
                if j == 7:
                    work.extend([proj_partial(st, 1) for st in range(4, ST)])
                for m in range(ST):
                    scores_exp(j, m)
                    if m == 5 and j > 0:
                        # at m=5 the scores are narrow (1 chunk per head),
                        # so rb_norm's psS/ACT usage doesn't contend with
                        # the wide m=4 step
                        rb_norm(j - 1)
                    if m == 0:
                        pend[f"ps{j}"] = [
                            psAV.tile([65, S], F32, tag="av",
                                      name=f"av{j}_{hb}") for hb in range(2)]
                    if m >= 2:
                        av_m(j, m - 2)
                    if m % 2 == 1 and work:
                        # drain evenly over the remaining odd slots so no
                        # burst of quanta piles up at the pair boundary
                        slots_left = (ST - m + 1) // 2
                        npop = -(-len(work) // slots_left)
                        for _ in range(npop):
                            if work:
                                work.pop(0)()
                av_m(j, ST - 2)
                while work:
                    work.pop(0)()
                av_m(j, ST - 1)
                evict_recip(j)
            rb_norm(NPAIRS - 1)

            # ---------- output projection: finish ko 3..7 ----------
            # The SBUF-resident partial is added (DVE, in place) to the
            # remaining-ko result and DMA'd out.
            for st in range(ST):
                for nE in range(2):
                    # alternate psum pools (psAV is dead now) so up to 8
                    # groups' ko3..6 matmuls pipeline while rb_norm(7)
                    # still gates every group's ko7
                    pool = psS if (st + nE) % 2 == 0 else psAV
                    ps = pool.tile([P, 512], F32,
                                   tag="ps" if pool is psS else "av",
                                   name=f"yps{st}_{nE}")
                    for ko in range(KPART, KO):
                        nc.tensor.matmul(
                            ps[:], outT[:, ko, st * P:(st + 1) * P],
                            xT[:, ko, nE * 512:(nE + 1) * 512],
                            start=(ko == KPART), stop=(ko == KO - 1))
                    yp = ypart_sb[(st, nE)]
                    nc.vector.tensor_add(yp[:], yp[:], ps[:])
                    nc.sync.dma_start(
                        y_d[st * P:(st + 1) * P, nE * 512:(nE + 1) * 512],
                        yp[:])

    nc.compile()
    return nc


def kernel(x, w_attn, b_attn, w_proj, b_proj):
    import ml_dtypes
    import concourse.bass_utils as bass_utils

    if "nc" not in _CACHE:
        _CACHE["nc"] = _build()
    nc = _CACHE["nc"]

    BF = ml_dtypes.bfloat16
    x = np.asarray(x, dtype=np.float32)
    w_attn = np.asarray(w_attn, dtype=np.float32)
    b_attn = np.asarray(b_attn, dtype=np.float32)
    w_proj = np.asarray(w_proj, dtype=np.float32)
    b_proj = np.asarray(b_proj, dtype=np.float32)

    xT = np.ascontiguousarray(
        np.transpose(x, (0, 2, 1)).astype(BF))                   # [B, D, S]
    # [D, 2D] -> [p, m, ko, e] so each m-tile is contiguous per partition
    wqkT = np.ascontiguousarray(
        w_attn[:2 * D].T.reshape(KO, P, MT, P).transpose(1, 2, 0, 3)
        .astype(BF))
    wvT = np.ascontiguousarray(w_attn[2 * D:].T.astype(BF))      # [D, D]
    wpT = np.ascontiguousarray(w_proj.T.astype(BF))              # [D, D]
    bqk = np.ascontiguousarray(b_attn[:2 * D])
    bv = b_attn[2 * D:]
    beff = (b_proj.astype(np.float64)
            + w_proj.astype(np.float64) @ bv.astype(np.float64)
            ).astype(np.float32)
    umask = np.triu(np.ones((P, P), dtype=np.float32))           # f >= p
    in_maps = [
        dict(xT=xT[b], wqkT=wqkT, wvT=wvT, wpT=wpT, bqk=bqk, beff=beff,
             umask=umask)
        for b in range(B)
    ]
    res = bass_utils.run_bass_kernel_spmd(
        nc, in_maps, core_ids=list(range(NCORES)), trace=TRACE)
    if TRACE:
        _CACHE["exec_time_ns"] = res.exec_time_ns
        _CACHE["trace"] = res.instructions_and_trace
    return np.stack([res.results[b]["y"] for b in range(B)], axis=0)
